# revision 48
# baseline (speedup 1.0000x reference)
"""ArcFace (AngularPenaltySMLoss) over x[4096, 32000] f32 on 8 TRN2 NeuronCores.

Data-parallel over the batch axis: each core gets 512 rows, processed as 4
blocks of 128 rows (partition dim). The host ships x as bf16 (halves DMA
bytes); all reductions accumulate in f32 on device.

Per-core structure (triple-buffered [128, 32000] bf16 tiles, 8 DMA chunks of
4000 cols):
  - x[row, target[row]] comes from 4 indirect SWDGE DMA gathers straight out
    of HBM (128 x 2B descriptors each), issued at kernel start -- they are
    independent of the block pipeline (replaces the old ~44us/block GPSIMD
    ap_gather).
  - sum of squares per row: custom-DVE TENSOR_TENSOR_REDUCE per chunk on DVE,
    with a tunable per-block set of chunks done as ACT activation(Square,
    accum_out) to balance engine load.
  - S/||row|| = exp(-0.5*ln(ssq) + ln(S)); Ln/Exp/Square all live in the
    natural_log_exp_and_others ACT table set -- the table list handed to the
    chooser is filtered so exactly one set serves every activation (the
    default greedy chooser alternates exp_and_others/natural_log, costing 12
    table loads and ACT stalls).
  - row sum of exp(S*x/||row||): ACT activation(Exp, scale=per-row AP,
    accum_out), software-pipelined one block behind the square pass and
    emitted BEFORE the current block's norm chain so the in-order ACT queue
    never head-of-line blocks on an unfinished norm. The LAST block's exp is
    split: half its chunks run as a polynomial exp on DVE (EXP_SQSUM_ANT,
    ((a*t+b)^2+c)^2, registered dynamically) so both engines drain together,
    with the last block's norm chain threaded between exp(prev) batches.
  - rowsum partials land in per-block columns and are reduced by ONE 3D
    tensor_reduce in the epilogue -- no mid-pipeline cross-engine reduce ever
    blocks the DVE queue.
Epilogue (batched over [128, 4]) computes
  num = S*(cos(M)*ct - sin(M)*sqrt(1-ct^2)), L = num - ln(exp(num)+rowsum-exp(S*ct))
and DMAs per-row L out; the host sums the 8 x [128, 4] partials into -mean(L).

(An attempted hand-authored 2X_1PORT packed-uop square op is parked behind
_build_graph(sq2x=True): its table generates and the REGULAR path runs, but
engaging perf mode 1 hangs the DVE engine -- do not enable.)
"""

import math

import ml_dtypes
import numpy as np

from operator import add as _op_add

import concourse.bacc as bacc
import concourse.bass as bass
import concourse.dve_ops as dve_ops
import concourse.mybir as mybir
import concourse.tile as tile
from concourse.bass_utils import run_bass_kernel_spmd
from concourse.dve_ops import TENSOR_TENSOR_REDUCE as CDVE_TTR
from concourse.dve_spec import C0, C1, C2, Spec, Src0, Zero, lower
from concourse.dve_spec import sq as _spec_sq
from concourse.dve_uop import (
    ENABLE,
    AluInp,
    AluOp,
    DelayInp,
    DveOpSpec,
    InpSel,
    OutPath,
    OutSel,
    Trigger,
    UopConfig,
)

N, C = 4096, 32000
NCORES = 8
RPC = N // NCORES          # rows per core = 512
P = 128                    # partitions (rows per block)
NBLK = RPC // P            # 4 blocks per core
DW = 4000                  # square-pass chunk width (matches DMA chunks)
NCH = C // DW              # 8 chunks per block
EW = 16000                 # ACT exp chunk width (dummy out rewinds over a
                           # 4000-wide scratch via a stride-0 outer AP dim)
NECH = C // DW             # rowsum partial columns per block (4000-granular)

S = 30.0
MARGIN = 0.3
EPS = 1e-7

XDT = mybir.dt.bfloat16
NPXDT = ml_dtypes.bfloat16

_GRAPH_CACHE = {}

# Polynomial exp for the DVE (5 ALU stages, 1 elem/cycle):
#   e^t ~= ((ALPHA*(t/2) + BETA)^2 + GAMMA)^2   for |t| <~ 1.1
# (minimax fit of a quadratic to e^v on [-0.55, 0.55], then squared; per-row
# scale C0 = ALPHA*S/(2*||row||) folds the normalization in). Max rel err
# 1.4e-2 at |t|=1.1, ~7e-4 mean; measured end-to-end loss impact < 3e-4.
EXP_ALPHA = 0.7003877289210781
EXP_BETA = 0.7401068664293284
EXP_GAMMA = 0.4549723008674721


def _ref_exp_sqsum(in0, in1, c0, c1, c2):
    import numpy as _np

    b = (_np.square(in0.astype(_np.float32) * c0 + c1) + c2) ** 2
    return b, b.reshape(b.shape[0], -1).sum(axis=-1, keepdims=True)


def _get_exp_op():
    """Register (once) and return the custom DVE op
    out = ((in0*c0 + c1)^2 + c2)^2, accum_out = sum(out)."""
    name = "EXP_SQSUM_ANT"
    for op in dve_ops.OPS:
        if op.name == name:
            return op
    spec = Spec(
        body=_spec_sq(_spec_sq(Src0 * C0 + C1) + C2),
        accum=_op_add,
        accum_init=Zero,
        reference=_ref_exp_sqsum,
    )
    op = dve_ops.DveOp(name, spec, subdim=False, uops_sha={})
    dve_ops.OPS.append(op)
    dve_ops.CUSTOM_DVE_SPECS[name] = spec
    dve_ops._SUB_OPCODE_FOR_NAME[name] = (
        max(dve_ops._SUB_OPCODE_FOR_NAME.values()) + 1
    )
    # pin the uops sha self-consistently (DveOp.compile checks it)
    for ver in ("v3", "v4"):
        r = DveOpSpec(
            name=name,
            opcode=dve_ops.get_dve_sub_opcode(name),
            uops=lower(spec, ver=ver),
            rd1_en=False,
        )
        op.uops_sha[ver] = r.sha(ver)
    return op

# Which square chunks run on ACT (rest on DVE), per block. DVE is the longer
# pole, so it gets the earliest chunks (starts right behind the first DMA);
# ACT takes tails. Which exp chunks run on ACT (rest on DVE via the poly op):
# the last block's exp is split so both engines drain together.
# Totals tuned so ACT busy ~= DVE busy (ACT ~3.7us/chunk, DVE ~4.3us/chunk).
ACT_SQ = {0: (4, 5, 6, 7), 3: (6, 7)}
# exp-chunk indices (at EW granularity) on ACT; the rest run as 4000-wide
# poly-exp on DVE. Block 3's second half drains on DVE.
ACT_EXP = {0: (0, 1), 1: (0, 1), 2: (0, 1), 3: (0,)}
# exp(prev) chunks emitted on ACT before the last block's norm chain slots in
TAIL_SPLIT = 1
PERF_MAX = 1


def _unified_act_tables(arch):
    """Table list for the act-table chooser with Exp/Ln/Square removed from
    every set except natural_log_exp_and_others, so the greedy chooser is
    forced onto the one set that serves all three (positions, and hence
    act_func_set_ids, are unchanged)."""
    from concourse.hw_specs import get_activation_tables

    keep = {"Exp", "Ln", "Square"}
    out = {}
    for name, fns in get_activation_tables(arch).items():
        if name != "natural_log_exp_and_others":
            fns = {f for f in fns if f.name not in keep}
        out[name] = set(fns)
    return out


def _ref_sqsum(in0, in1, c0, c1, c2):
    import numpy as _np

    b = _np.square(in0.astype(_np.float32))
    return b, b.reshape(b.shape[0], -1).sum(axis=-1, keepdims=True)


def _build_sq2x_uops():
    """Hand-authored 2X_1PORT program for out=in0^2, accum_out=sum(out).

    Each cycle reads two packed bf16 elements (SRC_0 lo / SRC_0_HI hi):
      dp0: lo2 = lo*lo                (delay1 <- hi)
      dp1: hi2 = hi*hi                (delay0 <- lo2)
      dp2: pair = hi2 + lo2           (delay0 <- lo2 carried, delay1 <- hi2)
      dp3: acc  = acc + pair          (a-port threading from here down)
      dp4..7: pass-through, carrying lo2/hi2 to the write muxes
    write0_lo = DELAY_0 (lo2), write0_hi = DELAY_1 (hi2).
    """
    P5 = DelayInp.PREV_DELAY

    # -- seed state: load Zero into the stage-3 accumulator flop
    seed = UopConfig()
    seed.accum_enabled = ENABLE
    seed.repeat_count = 1
    seed.trigger = (Trigger.COUNT, Trigger.NONE, Trigger.NONE)
    seed.next_uop = (1, 0, 0)
    seed.enable_input(InpSel.ZERO, 1).enable_input(InpSel.ZERO, 2)
    dp = seed.datapath_config
    dp[0].enable_delay_from_src(P5, 1)             # delay1 <- lane2 (Zero)
    dp[1].pass_through_delay(1)
    dp[2].pass_through_delay(1)
    dp[3].enable_alu(AluOp.BYPASS, AluInp.PREV_DELAY_1, AluInp.PREV_DELAY_1)
    dp[3].alu_out_a_enable = ENABLE
    for k in range(4, 8):
        dp[k].enable_alu(AluOp.BYPASS, AluInp.PREV_ALU_OUT)
        dp[k].alu_out_a_enable = ENABLE

    # -- steady state (lane wiring mirrors the stock TT 2X_1PORT program:
    # SRC_0 on lane 0 feeding block 0's ALU, SRC_0_HI on lane 2 -> delay1)
    st = UopConfig()
    st.accum_enabled = ENABLE
    st.trigger = (Trigger.SRC_TENSOR_DONE, Trigger.NONE, Trigger.NONE)
    st.require_inp0 = ENABLE
    st.enable_input(InpSel.SRC_0, 0).enable_input(InpSel.SRC_0_HI, 2)
    st.enable_output(OutSel.DELAY_0, OutPath.WR0_LO)
    st.enable_output(OutSel.DELAY_1, OutPath.WR0_HI)
    dp = st.datapath_config
    dp[0].enable_alu(AluOp.MULTIPLY, AluInp.PREV_ALU_OUT, AluInp.PREV_ALU_OUT)
    dp[0].enable_delay_from_src(P5, 1)             # delay1 <- lane2 (hi)
    dp[1].enable_alu(AluOp.MULTIPLY, AluInp.PREV_DELAY_1, AluInp.PREV_DELAY_1)
    dp[1].enable_delay_from_src(DelayInp.PREV_ALU_OUT, 0)   # delay0 <- lo2
    dp[2].enable_alu(AluOp.ADD, AluInp.PREV_ALU_OUT, AluInp.PREV_DELAY_0)
    dp[2].pass_through_delay(0)                    # keep lo2
    dp[2].enable_delay_from_src(DelayInp.PREV_ALU_OUT, 1)   # delay1 <- hi2
    dp[3].enable_alu(AluOp.ADD, AluInp.CURR_ALU_OUT, AluInp.PREV_ALU_OUT)
    dp[3].alu_out_a_enable = ENABLE
    dp[3].pass_through_delay(0, 1)
    for k in range(4, 8):
        dp[k].enable_alu(AluOp.BYPASS, AluInp.PREV_ALU_OUT)
        dp[k].alu_out_a_enable = ENABLE
        dp[k].pass_through_delay(0, 1)
    return [seed, st]


class _Sq2xOp:
    """DveOp-compatible wrapper whose compiled table carries a hand-built
    2X_1PORT uop program (perf_max=1). Semantics (CoreSim / IR tracing) come
    from the 1x Spec; the engine picks the packed program when the AP
    conditions allow and the instruction's perf_max byte enables it."""

    name = "SQSUM2X_ANT"
    subdim = False
    perf_en = {}

    def __init__(self):
        self.spec = Spec(
            body=_spec_sq(Src0),
            accum=_op_add,
            accum_init=Zero,
            reference=_ref_sqsum,
        )
        self._cache = {}

    def compile(self, ver):
        if ver not in self._cache:
            self._cache[ver] = DveOpSpec(
                name=self.name,
                opcode=dve_ops.get_dve_sub_opcode(self.name),
                uops=lower(self.spec, ver=ver),
                uops_2x=_build_sq2x_uops(),
                perf_max=1,
                rd1_en=False,
            )
        return self._cache[ver]


def _get_sq2x_op():
    name = _Sq2xOp.name
    for op in dve_ops.OPS:
        if op.name == name:
            return op
    op = _Sq2xOp()
    dve_ops.OPS.append(op)
    dve_ops.CUSTOM_DVE_SPECS[name] = op.spec
    dve_ops._SUB_OPCODE_FOR_NAME[name] = (
        max(dve_ops._SUB_OPCODE_FOR_NAME.values()) + 1
    )
    return op


def _build_graph(repeat=1, act_sq=None, act_exp=None, tail_split=TAIL_SPLIT,
                 bufs=3, ew=EW, sq2x=False, scr8=False):
    act_sq = ACT_SQ if act_sq is None else act_sq
    act_exp = ACT_EXP if act_exp is None else act_exp
    exp_op = _get_exp_op()
    sq_op = _get_sq2x_op() if sq2x else None
    nech = C // ew
    f32 = mybir.dt.float32
    AF = mybir.ActivationFunctionType
    OP = mybir.AluOpType
    AX = mybir.AxisListType

    nc = bacc.Bacc(
        "TRN2", target_bir_lowering=False, debug=False, num_devices=NCORES
    )
    # Route the act-table chooser through the filtered list (one set total).
    nc.insert_act_table_loads = _patched_insert_act_table_loads.__get__(nc)

    x_d = nc.dram_tensor("x", [RPC, C], XDT, kind="ExternalInput")
    idx_d = nc.dram_tensor("idx", [P, NBLK], mybir.dt.int32, kind="ExternalInput")
    out_d = nc.dram_tensor("out", [P, NBLK], f32, kind="ExternalOutput")

    with tile.TileContext(nc) as tc:
        with (
            tc.tile_pool(name="xbuf", bufs=bufs) as xpool,
            tc.tile_pool(name="small", bufs=1) as sp,
        ):
            idx_t = sp.tile([P, NBLK], mybir.dt.int32)
            ssq_part = sp.tile([P, NCH * 2], f32)
            # per-block columns (no parity reuse): all rowsum reductions
            # happen once in the epilogue, so no mid-pipeline reduce ever
            # blocks the DVE queue behind ACT's exp ladder. Fixed 8 cols per
            # block at 4000-granularity; wide (8000) ACT chunks use every
            # other column, the rest stay zero from the one-time memset.
            rs_part = sp.tile([P, NECH * NBLK], f32)
            lnv = sp.tile([P, NBLK], f32)       # ln(sum(x^2)) per row
            g32 = sp.tile([P, NBLK], f32)       # x[row, target[row]] (SWDGE casts)
            rowsum = sp.tile([P, NBLK], f32)    # sum(exp(S*xn)) per row
            ssq_b = sp.tile([P, 1], f32)
            cl_b = sp.tile([P, 1], f32)
            inv_s = sp.tile([P, NBLK], f32)     # S / ||row|| per block
            inv_p = sp.tile([P, NBLK], f32)     # ALPHA*S/(2||row||) for poly exp
            # scratch outs must live in SBUF: walrus' birverifier rejects
            # non-matmul/memset writes to PSUM. act_scr stays 4000 wide even
            # for wider exp chunks (stride-0 outer-dim rewind on the out AP).
            act_scr = sp.tile([P, DW], mybir.dt.float8e4 if scr8 else XDT)
            dve_scr = sp.tile([P, DW], XDT)
            ln_s = sp.tile([P, 1], f32)

            nc.gpsimd.memset(ln_s[:, :], float(math.log(S)))
            nc.gpsimd.memset(rs_part[:, :], 0.0)
            nc.sync.dma_start(idx_t[:, :], idx_d[:, :])

            x_flat = x_d[:, :].rearrange("r (c u) -> (r c) u", u=1)

            def emit_gathers():
                # target-column gather straight from HBM: one element per
                # partition via per-partition flat indices
                for b in range(NBLK):
                    nc.gpsimd.indirect_dma_start(
                        out=g32[:, b : b + 1],
                        out_offset=None,
                        in_=x_flat,
                        in_offset=bass.IndirectOffsetOnAxis(
                            ap=idx_t[:, b : b + 1], axis=0
                        ),
                    )

            sub = ew // DW  # 4000-wide subchunks per exp chunk
            if sub > 1:
                # never-read dummy out: rewind over the same 4000-wide
                # scratch via a stride-0 outer dim (halves ACT instr count
                # without more SBUF)
                act_out = (
                    act_scr[:, :DW]
                    .rearrange("p (u c) -> p u c", u=1)
                    .to_broadcast([P, sub, DW])
                )
            else:
                act_out = act_scr[:, :DW]

            def emit_exp_chunks(xt, b, chunks):
                # row-sum partials of exp(S * x / ||row||) for block b;
                # ACT-assigned chunks use the exp table, the rest run the
                # poly approximation on DVE (always 4000-wide)
                po = b * NECH
                on_act = set(act_exp.get(b, range(nech)))
                for c in chunks:
                    col0 = po + c * sub
                    if c in on_act:
                        nc.scalar.activation(
                            act_out,
                            xt[:, c * ew : (c + 1) * ew],
                            AF.Exp,
                            scale=inv_s[:, b : b + 1],
                            accum_out=rs_part[:, col0 : col0 + 1],
                        )
                    else:
                        for j in range(sub):
                            lo = c * ew + j * DW
                            nc.vector._custom_dve(
                                exp_op,
                                out=dve_scr[:, :],
                                in0=xt[:, lo : lo + DW],
                                s0=inv_p[:, b : b + 1],
                                s1=EXP_BETA,
                                imm2=EXP_GAMMA,
                                accum_out=rs_part[:, col0 + j : col0 + j + 1],
                            )

            def emit_rowsums():
                # one 3D reduce: [P, NBLK, NECH] -> [P, NBLK]
                nc.vector.tensor_reduce(
                    out=rowsum[:, :],
                    in_=rs_part[:, :].rearrange("p (b c) -> p b c", c=NECH),
                    axis=AX.X,
                    op=OP.add,
                )

            def emit_squares(xt, b):
                rows = slice(b * P, (b + 1) * P)
                on_act = set(act_sq.get(b, ()))
                po = (b % 2) * NCH
                for c in range(NCH):
                    cols = slice(c * DW, (c + 1) * DW)
                    nc.sync.dma_start(xt[:, cols], x_d[rows, cols])
                    if c in on_act:
                        nc.scalar.activation(
                            act_scr[:, :DW],
                            xt[:, cols],
                            AF.Square,
                            accum_out=ssq_part[:, po + c : po + c + 1],
                        )
                    elif sq_op is not None:
                        bi = nc.vector._custom_dve(
                            sq_op,
                            out=dve_scr[:, :],
                            in0=xt[:, cols],
                            accum_out=ssq_part[:, po + c : po + c + 1],
                        )
                        bi.ins.perf_max = PERF_MAX
                    else:
                        nc.vector._custom_dve(
                            CDVE_TTR,
                            out=dve_scr[:, :],
                            in0=xt[:, cols],
                            in1=xt[:, cols],
                            s0=0.0,
                            s1=1.0,
                            accum_out=ssq_part[:, po + c : po + c + 1],
                        )

            def emit_norm(b, need_poly):
                # per-block scale: inv_s = exp(-0.5*ln(ssq)+ln(S)) = S/sqrt(ssq)
                po = (b % 2) * NCH
                nc.vector.tensor_reduce(
                    out=ssq_b[:, :],
                    in_=ssq_part[:, po : po + NCH],
                    axis=AX.X,
                    op=OP.add,
                )
                nc.vector.tensor_scalar_max(cl_b[:, :], ssq_b[:, :], 1e-24)
                nc.scalar.activation(lnv[:, b : b + 1], cl_b[:, :], AF.Ln)
                nc.scalar.activation(
                    inv_s[:, b : b + 1],
                    lnv[:, b : b + 1],
                    AF.Exp,
                    bias=ln_s[:, :],
                    scale=-0.5,
                )
                if need_poly:
                    nc.vector.tensor_scalar_mul(
                        inv_p[:, b : b + 1],
                        inv_s[:, b : b + 1],
                        EXP_ALPHA / 2.0,
                    )

            def body():
                emit_gathers()
                allch = list(range(nech))
                last = NBLK - 1
                prev = None
                for b in range(NBLK):
                    xt = xpool.tile([P, C], XDT, tag="xt", name=f"xt{b}")
                    emit_squares(xt, b)
                    if b < last:
                        # deferred exp of the previous block, emitted before
                        # this block's norm chain so the in-order ACT queue
                        # never head-of-line blocks on an unfinished norm
                        if prev is not None:
                            emit_exp_chunks(prev[0], prev[1], allch)
                        emit_norm(b, need_poly=len(act_exp.get(b, allch)) < nech)
                    else:
                        # last block: thread its norm chain between exp(prev)
                        # batches so neither engine head-of-line blocks, then
                        # split exp(last) across ACT and DVE(poly) to drain
                        # both engines together
                        emit_exp_chunks(prev[0], prev[1], allch[:tail_split])
                        emit_norm(b, need_poly=len(act_exp.get(b, allch)) < nech)
                        emit_exp_chunks(prev[0], prev[1], allch[tail_split:])
                    prev = (xt, b)
                emit_exp_chunks(prev[0], prev[1], allch)
                emit_rowsums()

                # batched epilogue over [P, NBLK]
                inv_n = sp.tile([P, NBLK], f32, tag="ep_inv_n", name="ep_inv_n")
                ct = sp.tile([P, NBLK], f32, tag="ep_ct", name="ep_ct")
                e2 = sp.tile([P, NBLK], f32, tag="ep_e2", name="ep_e2")
                ctc = sp.tile([P, NBLK], f32, tag="ep_ctc", name="ep_ctc")
                sq = sp.tile([P, NBLK], f32, tag="ep_sq", name="ep_sq")
                lnom = sp.tile([P, NBLK], f32, tag="ep_lnom", name="ep_lnom")
                sn = sp.tile([P, NBLK], f32, tag="ep_sn", name="ep_sn")
                a1 = sp.tile([P, NBLK], f32, tag="ep_a1", name="ep_a1")
                b1 = sp.tile([P, NBLK], f32, tag="ep_b1", name="ep_b1")
                num = sp.tile([P, NBLK], f32, tag="ep_num", name="ep_num")
                e1 = sp.tile([P, NBLK], f32, tag="ep_e1", name="ep_e1")
                den = sp.tile([P, NBLK], f32, tag="ep_den", name="ep_den")
                lden = sp.tile([P, NBLK], f32, tag="ep_lden", name="ep_lden")
                lt = sp.tile([P, NBLK], f32, tag="ep_lt", name="ep_lt")

                nc.scalar.activation(inv_n[:, :], lnv[:, :], AF.Exp, scale=-0.5)
                nc.vector.tensor_tensor(ct[:, :], g32[:, :], inv_n[:, :], OP.mult)
                nc.scalar.activation(e2[:, :], ct[:, :], AF.Exp, scale=S)
                nc.vector.tensor_scalar(
                    ctc[:, :], ct[:, :], -1.0 + EPS, 1.0 - EPS, OP.max, OP.min
                )
                nc.vector.tensor_tensor(sq[:, :], ctc[:, :], ctc[:, :], OP.mult)
                # ln(1 - ctc^2) via the activation's free affine: -1*sq + 1
                nc.scalar.activation(lnom[:, :], sq[:, :], AF.Ln, bias=1.0, scale=-1.0)
                nc.scalar.activation(sn[:, :], lnom[:, :], AF.Exp, scale=0.5)
                nc.vector.tensor_scalar_mul(a1[:, :], ctc[:, :], S * math.cos(MARGIN))
                nc.vector.tensor_scalar_mul(b1[:, :], sn[:, :], S * math.sin(MARGIN))
                nc.vector.tensor_tensor(num[:, :], a1[:, :], b1[:, :], OP.subtract)
                nc.scalar.activation(e1[:, :], num[:, :], AF.Exp)
                # rowsum - e2 computed off the critical chain (e2 is ready
                # well before num/e1), so only one add remains on it
                nc.vector.tensor_tensor(den[:, :], rowsum[:, :], e2[:, :], OP.subtract)
                nc.vector.tensor_tensor(den[:, :], den[:, :], e1[:, :], OP.add)
                nc.scalar.activation(lden[:, :], den[:, :], AF.Ln)
                nc.vector.tensor_tensor(lt[:, :], num[:, :], lden[:, :], OP.subtract)
                nc.sync.dma_start(out_d[:, :], lt[:, :])

            if repeat == 1:
                body()
            else:
                with tc.For_i(0, repeat, 1):
                    body()

    nc.compile()
    return nc


def _patched_insert_act_table_loads(self):
    import bass_rust as _bass_rust

    has_activation = any(
        isinstance(i, mybir.InstActivation)
        for b in self.main_func.blocks
        for i in b.instructions
    )
    if not has_activation:
        return
    tables = list(_unified_act_tables(self.m.arch).items())
    _bass_rust.insert_act_table_loads(self, tables)


def get_graph():
    if "nc" not in _GRAPH_CACHE:
        _GRAPH_CACHE["nc"] = _build_graph()
    return _GRAPH_CACHE["nc"]


def make_in_maps(x, target):
    x = np.asarray(x, dtype=np.float32)
    xq = np.ascontiguousarray(x.astype(NPXDT))
    tgt = np.asarray(target).astype(np.int64).reshape(N)
    in_maps = []
    for i in range(NCORES):
        ts = tgt[i * RPC : (i + 1) * RPC].reshape(NBLK, P)  # [NBLK, P]
        # flat element index of x[row, target[row]] within this core's shard
        rowbase = (
            np.arange(NBLK)[:, None] * P + np.arange(P)[None, :]
        ) * C  # [NBLK, P]
        idx = (rowbase + ts).astype(np.int32).T  # [P, NBLK]
        in_maps.append(
            {
                "x": xq[i * RPC : (i + 1) * RPC],
                "idx": np.ascontiguousarray(idx),
            }
        )
    return in_maps


def run(x, target, **spmd_kwargs):
    import time

    nc = get_graph()
    in_maps = make_in_maps(x, target)
    last_err = None
    for attempt in range(3):
        try:
            res = run_bass_kernel_spmd(
                nc, in_maps, core_ids=list(range(NCORES)), **spmd_kwargs
            )
            break
        except Exception as e:  # transient fleet/device errors observed
            last_err = e
            time.sleep(3.0)
    else:
        raise last_err
    total = 0.0
    for r in res.results:
        total += float(np.asarray(r["out"], dtype=np.float64).sum())
    return np.asarray(-(total / N), dtype=np.float32), res


def kernel(x, target):
    loss, _ = run(x, target)
    return loss
